# revision 1
# baseline (speedup 1.0000x reference)
"""Trainium2 Bass kernel for nn_MultiHeadAttentionQuantum.

Math (verified vs reference to ~6e-7 rel):
  - _qlayer(x, phi)[t, w] reduces to prefix products of cos(x+phi):
      out[t, w]   = prod_{j<=w} cos(x[t,j]+phi[j])   (w >= 1)
      out[t, 0]   = prod_{j=1..7} cos(x[t,j]+phi[j])
    (RX rotations + CNOT-ring = prefix-XOR => Z-expectations = cos products.)
  - QuantumKernel sim factorizes rank-16:
      sim[i,j] = prod_{w<4} cos((q_iw - k_jw)/2) = F_i . G_j,
      F_m = prod_w (cos(q_w/2) or sin(q_w/2)) by bits of m; same for G with k.
    q, k in [-1, 1] => (q-k)/2 in [-1, 1] => every cos factor > 0 => the
    reference's abs() is a no-op.
  - softmax without max-subtraction (sim in [0,1], exp in [1, e]):
      acc = E @ [v | 1]  -> rows 0..7 numerator, row 8 denominator;
      a final 9x9 matmul applies W and adds bias via the b*den trick;
      the division happens in token-major layout.

Sharding: data-parallel over batch B=8, one batch per NeuronCore, no
collectives. Full inputs in, full output out; host only slices/stacks.

Layout per core ("linear split"): SBUF partition p holds tokens
16p..16p+15 (contiguous 512B DMA lines both directions). Token group
a = {16p+a : p} is a column slice everywhere, so the internal
key/query permutation is self-consistent and cancels out.

Performance structure (cost-model 51.9us, vs 57.5us f32r baseline;
walrus-verified and validated on the 8-core HW at 1.51e-2 rel vs the
2e-2 budget):
  - features F,G quantized to fp8e4m3 (staged/transposed in bf16 --
    fp8 PE transposes need 2-byte output steps) and stored [9, 2, S]
    so the sim matmul runs in DoubleRow perf mode (0.5 PE cycles/row,
    2x f32r).  Row 8 is a constant 17th feature (F=1, G=BETA, DMAed
    from a tiny host constant) so the matmul emits w = sim + BETA.
  - the exp of the [S,S] sim matrix -- the dominant elementwise work,
    ~39us if done on ACT alone -- is split across two lanes: ACT
    computes true exp(w) for 12 key-tiles per half-pass (the e^BETA
    scale cancels in softmax); DVE handles 4 tiles with the linear
    e^w ~= LIN_A*w + LIN_C (5.2% minimax rel err on w in [.585,1.5];
    one single-PSUM-read tensor_scalar).  Softmax normalization
    absorbs the oscillating fit error: measured 2.3e-3 in CoreSim,
    1.50e-2 on HW (HW activation/f32r numerics dominate).
    GPSIMD cannot touch PSUM and rejects TensorScalarPtr, so Pool only
    runs memsets/identity masks.
  - eps PSUM tiles are [128,512], 4 deep: with the ACT/DVE lanes
    alternating per kt, each lane owns a private double-buffer, so eps
    production never sits on a consumer lane's critical path.
  - lead-in: x DMA first; a dummy Sin loads the trig table at t~0 and
    a dummy Exp right after the last front-end Sin hoists the exp
    table switch off the critical path; the feature half-angle
    cos/sin(z/2) for chunk B use short-range polynomials (no ACT
    table); PSUM->SBUF feature copies are split DVE/ACT in the
    lead-in.
  - tail: w9 matmul in f32r; the two j-halves of each half-pass tail
    run as parallel chains (hp0 fully on DVE mid-stream; hp1 split
    DVE/ACT at the end), each DMAing its 32 output columns as soon as
    they are divided.

Matmuls run in float32r / fp8-DoubleRow (operand rounding enters via
the attention weights and averages out over 2048 keys).
"""
import os
import numpy as np

import concourse.bass as bass
import concourse.tile as tile
from concourse import bacc, mybir
from concourse.bass_utils import run_bass_kernel_spmd
from concourse.masks import make_identity

F32 = mybir.dt.float32
F32R = mybir.dt.float32r
F8 = mybir.dt.float8e4
BF16 = mybir.dt.bfloat16
DR = mybir.MatmulPerfMode.DoubleRow
AL = mybir.AluOpType
ACTF = mybir.ActivationFunctionType

B, S, E = 8, 2048, 8
P = 128          # SBUF partitions
G = 16           # token groups per partition (S / P)
NF = 16          # feature rank
MAGIC = 12582912.0           # 1.5 * 2**23: fp32 round-to-nearest trick
TWO_PI = float(2.0 * np.pi)
HALF_PI = float(0.5 * np.pi)
if os.environ.get("MM_DTYPE", "f32r") == "f32":
    F32R = F32

# The eps matmul computes w = sim + BETA via a constant 17th feature
# row (F=1, G=BETA; BETA=0.5 is exact in fp8).  ACT tiles exp it (the
# e^BETA scale cancels in softmax); offloaded tiles instead use the
# pure quadratic e^w ~= AQ*w^2 + CQ (max rel err 4e-3 on w in
# [0.585, 1.5]; a single one-PSUM-input pow op on DVE).  After softmax
# smoothing the end-to-end impact is ~1e-4 per offloaded tile.
BETA = 0.5
AQ = 1.3955493783
CQ = 1.3236611006
# 1-op offload: e^w ~= LIN_A*w + LIN_C on [0.585, 1.5] (5.2% max rel;
# softmax smoothing -> ~3e-3 end-to-end with 6/16 tiles).  Computed as
# a single one-PSUM-read tensor_scalar, so no vaug rescale and no GAM
# correction are needed.
LIN_A = 2.7843213923
LIN_C = 0.0734060804

# minimax quadratics in u = z^2 for the feature half-angle trig on
# z in [-1.05, 1.05] (so the cs Sins never touch the ACT trig table):
#   cos(z/2) ~= CA*(u - CR2) * (u - CR1)        (max abs err 9e-7)
#   sin(z/2) ~= z * (SA*(u + SC)^2 + SE)        (max abs err 6e-8)
CA = 0.0025684907
CR1 = 10.095342491
CR2 = 38.5656410369
SA = 0.00025786501435758427
SC = -40.393770153695726
SE = 0.0792527656613275


def _parse_off(env, default):
    v = os.environ.get(env, default)
    return frozenset(int(t) for t in v.split(",") if t != "")


# eps PSUM tiles are 4-deep, consumer lanes (ACT/DVE/Pool) interleave
# per kt; pair ACT exp tiles with offloaded quadratic tiles so the lanes
# run concurrently.  Per half-pass: Pool's tiles sit where Pool is free
# (it runs chunk-B + copies until ~16us), and each half ENDS on ACT
# tiles so the stream drains through the fastest lane.
DVE_OFF = [_parse_off("DVE_OFF0", "3,7,11,14"),
           _parse_off("DVE_OFF1", "4,8,11,13")]
POOL_OFF = [_parse_off("POOL_OFF0", ""),
            _parse_off("POOL_OFF1", "")]
OFF_ALL = [sorted(DVE_OFF[0] | POOL_OFF[0]), sorted(DVE_OFF[1] | POOL_OFF[1])]
# j-granular offload (2*kt+j indices); default = both j of DVE_OFF kts
OFFJ = [_parse_off("DVE_OFFJ0",
                   ",".join(str(2 * k + j) for k in DVE_OFF[0]
                            for j in (0, 1))),
        _parse_off("DVE_OFFJ1",
                   ",".join(str(2 * k + j) for k in DVE_OFF[1]
                            for j in (0, 1)))]

_NC_CACHE = {}


def _make_crow():
    """Constant feature rows: row 0 = F (1 for k-tile 0, 0 for k-tile
    1); row 1 = G (BETA / 0 per group, (a, i, t) layout)."""
    import ml_dtypes
    crow = np.zeros((2, 4096), dtype=ml_dtypes.float8_e4m3)
    crow[0, 0:2048] = 1.0
    g = crow[1].reshape(G, 2, 128)
    g[:, 0, :] = BETA
    return crow


def _cos_chain(nc, work, x_bc, phi_bc, n, tagp, ag=G, eng=None):
    """cos(x + phi) for n stacked phi-chains over ag token groups.
    x_bc / phi_bc: [P, n, ag, E] views (stride-0 broadcasts allowed).
    Elementwise ops run on `eng` (default DVE); the Sin runs on ACT.
    Returns c tile [P, n*ag*E]."""
    eng = eng or nc.vector
    W = n * ag * E
    psi = work.tile([P, W], F32, tag=f"psi{tagp}")
    eng.tensor_tensor(
        psi[:].rearrange("p (n a w) -> p n a w", n=n, a=ag), x_bc, phi_bc,
        op=AL.add)
    # range-reduce psi to [-pi, pi]:  psi - 2pi*round(psi/2pi)
    t1 = work.tile([P, W], F32, tag=f"t1{tagp}")
    eng.tensor_scalar(t1[:], psi[:], float(1.0 / TWO_PI), MAGIC,
                      op0=AL.mult, op1=AL.add)
    t2 = work.tile([P, W], F32, tag=f"t2{tagp}")
    eng.tensor_scalar(t2[:], t1[:], MAGIC, TWO_PI,
                      op0=AL.subtract, op1=AL.mult)
    red = work.tile([P, W], F32, tag=f"red{tagp}")
    eng.tensor_tensor(red[:], psi[:], t2[:], op=AL.subtract)
    c = work.tile([P, W], F32, tag=f"c{tagp}")
    nc.scalar.activation(c[:], red[:], ACTF.Sin)  # sin(x+phi+pi/2)=cos(x+phi)
    return c


def _prefix_products(nc, work, c3, n, tagp, ag=G, eng=None):
    """u[j] = c[j-1]*c[j] (j>=1, u[0]=c[0]); v[j] = prod c[max(0,j-3)..j],
    per chain/group. c3: [P, n, ag, 8] view."""
    eng = eng or nc.vector
    u = work.tile([P, n * ag * 8], F32, tag=f"u{tagp}")
    u3 = u[:].rearrange("p (n a w) -> p n a w", n=n, a=ag)
    eng.tensor_copy(u3[:, :, :, 0:1], c3[:, :, :, 0:1])
    eng.tensor_tensor(u3[:, :, :, 1:8], c3[:, :, :, 1:8],
                      c3[:, :, :, 0:7], op=AL.mult)
    v = work.tile([P, n * ag * 8], F32, tag=f"v{tagp}")
    v3 = v[:].rearrange("p (n a w) -> p n a w", n=n, a=ag)
    eng.tensor_copy(v3[:, :, :, 0:2], u3[:, :, :, 0:2])
    eng.tensor_tensor(v3[:, :, :, 2:8], u3[:, :, :, 2:8],
                      u3[:, :, :, 0:6], op=AL.mult)
    return u3, v3


def _suffix1(nc, work, u3, c3, out1, n, tagp, ag=G, eng=None):
    """out1 [P, n, ag, 1] <- prod c[1..7] = u2*u4*u6*c7."""
    eng = eng or nc.vector
    ta = work.tile([P, n * ag], F32, tag=f"s1a{tagp}")
    ta3 = ta[:].rearrange("p (n a) -> p n a", n=n).unsqueeze(3)
    eng.tensor_tensor(ta3, u3[:, :, :, 2:3], u3[:, :, :, 4:5],
                      op=AL.mult)
    tb = work.tile([P, n * ag], F32, tag=f"s1b{tagp}")
    tb3 = tb[:].rearrange("p (n a) -> p n a", n=n).unsqueeze(3)
    eng.tensor_tensor(tb3, u3[:, :, :, 6:7], c3[:, :, :, 7:8],
                      op=AL.mult)
    eng.tensor_tensor(out1, ta3, tb3, op=AL.mult)


def _build_nc(reps=1):
    nc = bacc.Bacc("TRN2", target_bir_lowering=False, debug=False,
                   num_devices=B)
    x_d = nc.dram_tensor("x", [S, E], F32, kind="ExternalInput").ap()
    w9_d = nc.dram_tensor("w9", [9, 9], F32, kind="ExternalInput").ap()
    phis_d = nc.dram_tensor("phis", [3, E], F32, kind="ExternalInput").ap()
    crow_d = nc.dram_tensor("crow", [2, 4096], F8, kind="ExternalInput").ap()
    out_d = nc.dram_tensor("out", [S, E], F32, kind="ExternalOutput").ap()

    with tile.TileContext(nc) as tc:
        with (
            tc.tile_pool(name="sb", bufs=1) as sb,
            tc.tile_pool(name="work", bufs=2) as work,
            tc.tile_pool(name="epool", bufs=4) as epool,
            tc.tile_pool(name="psb", bufs=3, space="PSUM") as psb,
            tc.tile_pool(name="psa", bufs=1, space="PSUM") as psa,
        ):
          for _rep in range(reps):
            # ---- trig-table prefetch: a dummy Sin as the very first ACT
            # op loads the trig table while the x DMA is still in flight --
            tw0 = sb.tile([1, 1], F32, tag="tw0")
            nc.gpsimd.memset(tw0[:], 0.0)
            tw1 = sb.tile([1, 1], F32, tag="tw1")
            nc.scalar.activation(tw1[:], tw0[:], ACTF.Sin)

            # ---- loads & constants (x first: it gates everything) ----
            x_sb = sb.tile([P, P], F32, tag="x")
            nc.sync.dma_start(
                x_sb[:], x_d.rearrange("(p a) w -> p (a w)", p=P))
            phib = sb.tile([P, 3 * E], F32, tag="phib")
            nc.sync.dma_start(
                phib[:],
                phis_d.rearrange("n w -> (n w)").unsqueeze(0)
                .broadcast_to((P, 3 * E)))
            w9_ld = sb.tile([9, 9], F32, tag="w9ld")
            nc.sync.dma_start(w9_ld[:], w9_d[:])
            w9_sb = sb.tile([9, 9], F32R, tag="w9")
            nc.vector.tensor_copy(w9_sb[:], w9_ld[:])
            ident = sb.tile([P, P], F32, tag="ident")
            make_identity(nc, ident[:])
            ident8 = sb.tile([P, P], BF16, tag="ident8")
            make_identity(nc, ident8[:])
            half_pi = sb.tile([P, 1], F32, tag="half_pi_const")
            nc.vector.memset(half_pi[:], HALF_PI)
            phibs = sb.tile([P, 3 * E], F32, tag="phibs")
            nc.vector.tensor_scalar(phibs[:], phib[:], HALF_PI, None,
                                    op0=AL.add)
            phibs3 = phibs[:].rearrange("p (n w) -> p n w", n=3)

            # ---- PE warm-up: dummy transposes while DVE runs the
            # front-end chain (keeps the PE p-state/HAM at full clock) ----
            pewarm = psb.tile([P, P], BF16, tag="small", bufs=2)
            for _ in range(int(os.environ.get('PEWARM', '6'))):
                nc.tensor.transpose(pewarm[:], ident8[:], ident8[:])

            # ---- fused q+k qlayer + features, emitted in two group
            # slices: a narrow chain (groups 0-3) unblocks the first
            # matmuls ~6us earlier; the rest overlaps the early loop ----
            x3 = x_sb[:].rearrange("p (a w) -> p a w", a=G)
            z4 = sb.tile([P, 2 * G * 4], F32, tag="z4")
            z44 = z4[:].rearrange("p (n a w) -> p n a w", n=2, a=G)
            feats = sb.tile([P, 2 * G * NF], BF16, tag="feats")
            feats5 = feats[:].rearrange("p (n a hi lo) -> p n a hi lo",
                                        n=2, a=G, hi=4)

            # full-width q+k cos chain (ONE ACT Sin), then chunked
            # post-work
            x_bc = x3.unsqueeze(1).broadcast_to((P, 2, G, E))
            phiqk = phibs3[:, 0:2, :].unsqueeze(2).broadcast_to((P, 2, G, E))
            c_qk = _cos_chain(nc, work, x_bc, phiqk, 2, "A", G)
            # v chain: elementwise on Pool, Sin on ACT (right after sin_qk)
            xv = x3.unsqueeze(1).broadcast_to((P, 1, G, E))
            phiv = phibs3[:, 2:3, :].unsqueeze(2).broadcast_to((P, 1, G, E))
            cv = _cos_chain(nc, work, xv, phiv, 1, "v", G)
            cqk3 = c_qk[:].rearrange("p (n a w) -> p n a w", n=2, a=G)
            _chunk_c = {"A1": (cqk3, 0), "A2": (cqk3, 0), "B": (cqk3, 0)}
            warm2 = psb.tile([P, P], F32, tag="small", bufs=2,
                             name="warm2")
            for _ in range(int(os.environ.get('PEWARM2', '4'))):
                nc.tensor.transpose(warm2[:], c_qk[:, 0:P], ident[:])
            _front = {}

            def front_ph1(a0, a1, tg, eng, cs_act=False):
                """prefix products -> z4 slice -> cs either via ACT Sins
                (parallel engine; needs trig table) or via short-range
                polynomials in u = z^2 on `eng`."""
                ag = a1 - a0
                cfull, cbase = _chunk_c[tg]
                c3 = cfull[:, :, a0 - cbase:a1 - cbase]
                u3, v3 = _prefix_products(nc, work, c3, 2, tg, ag, eng=eng)
                zs = z44[:, :, a0:a1, :]
                eng.tensor_copy(zs[:, :, :, 1:4], v3[:, :, :, 1:4])
                _suffix1(nc, work, u3, c3, zs[:, :, :, 0:1], 2, tg, ag,
                         eng=eng)
                if cs_act:
                    cs = work.tile([P, 2 * 2 * ag * 4], F32, tag=f"cs{tg}")
                    cs5 = cs[:].rearrange("p (b n a w) -> p b n a w",
                                          b=2, n=2, a=ag)
                    nc.scalar.activation(cs5[:, 0], zs, ACTF.Sin,
                                         bias=half_pi[:], scale=0.5)
                    nc.scalar.activation(cs5[:, 1], zs, ACTF.Sin, scale=0.5)
                    _front[tg] = (cs5, a0)
                    return
                # cs: [P, (b, n, a, w)]: b=0 cos(z/2), b=1 sin(z/2)
                W4 = 2 * ag * 4
                uz = work.tile([P, W4], F32, tag=f"uz{tg}")
                uz4 = uz[:].rearrange("p (n a w) -> p n a w", n=2, a=ag)
                eng.tensor_tensor(uz4, zs, zs, op=AL.mult)
                cs = work.tile([P, 2 * W4], F32, tag=f"cs{tg}")
                cs5 = cs[:].rearrange("p (b n a w) -> p b n a w",
                                      b=2, n=2, a=ag)
                # cos(z/2) = [CA*(u-CR2)] * (u-CR1)
                cw = work.tile([P, W4], F32, tag=f"cw{tg}")
                eng.tensor_scalar(cw[:], uz[:], -CR2, CA,
                                  op0=AL.add, op1=AL.mult)
                eng.scalar_tensor_tensor(
                    cs5[:, 0], uz4, -CR1,
                    cw[:].rearrange("p (n a w) -> p n a w", n=2, a=ag),
                    op0=AL.add, op1=AL.mult)
                # sin(z/2) = z * (SA*(u+SC)^2 + SE)
                sw = work.tile([P, W4], F32, tag=f"sw{tg}")
                eng.tensor_scalar(sw[:], uz[:], SC, None, op0=AL.add)
                sq = work.tile([P, W4], F32, tag=f"sq{tg}")
                eng.tensor_tensor(sq[:], sw[:], sw[:], op=AL.mult)
                sp = work.tile([P, W4], F32, tag=f"sp{tg}")
                eng.tensor_scalar(sp[:], sq[:], SA, SE,
                                  op0=AL.mult, op1=AL.add)
                eng.tensor_tensor(
                    cs5[:, 1], zs,
                    sp[:].rearrange("p (n a w) -> p n a w", n=2, a=ag),
                    op=AL.mult)
                _front[tg] = (cs5, a0)

            def front_ph2(a0, a1, key, tg=None, eng=None):
                """feature outer products from cs."""
                eng = eng or nc.vector
                tg = tg or key
                ag = a1 - a0
                cs5_full, base = _front[key]
                cs5 = cs5_full[:, :, :, a0 - base:a1 - base]

                def sel(w):
                    return cs5[:, :, :, :, w:w + 1].squeeze(4).transpose(
                        [0, 2, 3, 1])

                a01 = work.tile([P, 2 * ag * 4], F32, tag=f"a01{tg}")
                eng.tensor_tensor(
                    a01[:].rearrange("p (n a b1 b0) -> p n a b1 b0",
                                     n=2, a=ag, b1=2),
                    sel(0).unsqueeze(3).broadcast_to((P, 2, ag, 2, 2)),
                    sel(1).unsqueeze(4).broadcast_to((P, 2, ag, 2, 2)),
                    op=AL.mult)
                a23 = work.tile([P, 2 * ag * 4], F32, tag=f"a23{tg}")
                eng.tensor_tensor(
                    a23[:].rearrange("p (n a b3 b2) -> p n a b3 b2",
                                     n=2, a=ag, b3=2),
                    sel(2).unsqueeze(3).broadcast_to((P, 2, ag, 2, 2)),
                    sel(3).unsqueeze(4).broadcast_to((P, 2, ag, 2, 2)),
                    op=AL.mult)
                eng.tensor_tensor(
                    feats5[:, :, a0:a1, :, :],
                    a01[:].rearrange("p (n a lo) -> p n a lo", n=2, a=ag)
                          .unsqueeze(3).broadcast_to((P, 2, ag, 4, 4)),
                    a23[:].rearrange("p (n a hi) -> p n a hi", n=2, a=ag)
                          .unsqueeze(4).broadcast_to((P, 2, ag, 4, 4)),
                    op=AL.mult)

            front_ph1(0, 8, "A1", nc.vector, cs_act=True)
            nc.scalar.activation(tw0[:], tw1[:], ACTF.Exp)  # prefetch table
            featv = feats[:].rearrange("p (n am) -> p n am", n=2)

            # ---- transpose features to fp8 [8, 2, *] (DoubleRow k-tile
            # layout: feature m = p + 8i lives at partition p, k-tile i).
            # Emitted lazily: only the blocks the first matmuls need come
            # first; the rest interleave into the kt loop (PE gap filler).
            Ffeat8 = sb.tile([9, 2 * S], F8, tag="Ffeat8")    # (i, t)
            Gfeat8 = sb.tile([9, G * 256], F8, tag="Gfeat8")  # (a, i, t)
            Ffq = Ffeat8[:].rearrange("p (i t) -> p i t", i=2)
            Ff4 = Ffeat8[0:8].rearrange("p (i a t) -> p i a t", i=2, a=G)
            Gf4 = Gfeat8[:].rearrange("p (a i t) -> p a i t", a=G, i=2)
            # constant 17th feature: F row = (1, 0), G row = (BETA, 0) per
            # k-tile -> eps = sim + BETA.  Engines cannot address a bare
            # partition 8, so the rows come in via DMA from a tiny host
            # constant.
            nc.scalar.dma_start(Ffeat8[8:9, :], crow_d[0:1, :])
            nc.scalar.dma_start(Gfeat8[8:9, :], crow_d[1:2, :])

            def emit_tp_block(blk, mode="lead"):
                # 4 token groups of F features, split lo/hi into k-tiles;
                # mode picks who does the PSUM->SBUF move: 'lead' splits
                # DVE+ACT (ACT idle in the lead-in), 'dve' keeps both on
                # DVE (mid-stream; ACT is pacing), 'pool' likewise.
                tf = psb.tile([8, 1024], BF16, tag="small", bufs=2,
                              name=f"tf{blk}")
                for al in range(4):
                    a = blk * 4 + al
                    for i in range(2):
                        nc.tensor.transpose(
                            tf[:, (al * 2 + i) * P:(al * 2 + i + 1) * P],
                            featv[:, 0, a * NF + i * 8:a * NF + i * 8 + 8],
                            ident8[:])
                tf4 = tf[:].rearrange("p (al i t) -> p al i t", al=4, i=2)
                if mode == "lead":
                    nc.vector.tensor_copy(
                        Ff4[:, 0, blk * 4:(blk + 1) * 4, :], tf4[:, :, 0, :])
                    nc.scalar.copy(
                        Ff4[:, 1, blk * 4:(blk + 1) * 4, :], tf4[:, :, 1, :])
                else:
                    for i in range(2):
                        nc.vector.tensor_copy(
                            Ff4[:, i, blk * 4:(blk + 1) * 4, :],
                            tf4[:, :, i, :])

            def emit_tp_groups(a0, a1, dma=False):
                # token groups [a0, a1) of G features in one PSUM tile
                na = a1 - a0
                tg = psb.tile([8, 256 * na], BF16, tag="small", bufs=2,
                              name=f"tg{a0}")
                for al in range(na):
                    a = a0 + al
                    for i in range(2):
                        nc.tensor.transpose(
                            tg[:, (al * 2 + i) * P:(al * 2 + i + 1) * P],
                            featv[:, 1, a * NF + i * 8:a * NF + i * 8 + 8],
                            ident8[:])
                del dma  # GPSIMD cannot read PSUM on hw; always DVE
                nc.vector.tensor_copy(
                    Gfeat8[0:8, a0 * 256:a1 * 256], tg[:])

            # ---- qlayer for v: entirely on Pool (overlaps DVE front-end) --
            cv3 = cv[:].rearrange("p (n a w) -> p n a w", n=1, a=G)
            uv3, vv3 = _prefix_products(nc, work, cv3, 1, "v", G)
            vaug = sb.tile([P, G * 9], F32, tag="vaug")
            nc.gpsimd.memset(vaug[:], 1.0)          # col 8 of each group = 1
            va4 = vaug[:].rearrange("p (a w) -> p a w", a=G).unsqueeze(1)
            nc.vector.tensor_copy(va4[:, :, :, 1:4], vv3[:, :, :, 1:4])
            nc.vector.tensor_tensor(va4[:, :, :, 4:8], vv3[:, :, :, 4:8],
                                    vv3[:, :, :, 0:4], op=AL.mult)
            _suffix1(nc, work, uv3, cv3, va4[:, :, :, 0:1], 1, "v", G)
            vaug_r = sb.tile([P, G * 9], F32R, tag="vaug_r")
            nc.vector.tensor_copy(vaug_r[:], vaug[:])
            # scaled copy for quadratic-offloaded key tiles: E ~= CU*u + GAM;
            vaugc_r = sb.tile([P, G * 9], F32R, tag="vaugc_r")
            nc.vector.tensor_scalar(vaugc_r[:], vaug[:], AQ, None,
                                    op0=AL.mult)

            # gamma correction per half: g9 = GAM * sum_off(vaug_t)
            gones_f = sb.tile([P, 1], F32, tag="gones_f")
            nc.gpsimd.memset(gones_f[:], 0.0)
            g9sb = []
            for h in range(2):
                g9ps = psb.tile([9, 1], F32, tag="small", bufs=2,
                                name=f"g9ps{h}")
                for idx, kp in enumerate(OFF_ALL[h]):
                    nc.tensor.matmul(
                        g9ps[:], vaug[:, kp * 9:(kp + 1) * 9], gones_f[:],
                        start=(idx == 0), stop=(idx == len(OFF_ALL[h]) - 1))
                g9h = sb.tile([9, 1], F32, tag=f"g9sb{h}")
                if OFF_ALL[h]:
                    nc.vector.tensor_copy(g9h[:], g9ps[:])
                else:
                    nc.vector.memset(g9h[:], 0.0)
                g9sb.append(g9h)

            # ---- features + transposes, first-needed-first.
            # Chunk A (groups 0-8) stays on DVE; chunk B runs entirely on
            # Pool (its features are first needed at kt=8, ~8us later).
            front_ph2(0, 8, "A1")
            emit_tp_groups(0, 1)
            emit_tp_block(0)
            emit_tp_block(1)
            front_ph1(8, 16, "B", nc.vector)
            emit_tp_groups(1, 4, dma=True)
            emit_tp_groups(4, 8, dma=True)
            front_ph2(8, 16, "B")

            # ---- main loop: 2 query half-passes, pipelined over kt ----
            ftok = sb.tile([P, G * 9], F32, tag="ftok")
            recip = sb.tile([P, G], F32, tag="recip")
            outt = sb.tile([P, P], F32, tag="outt")
            out_v = out_d.rearrange("(p a) w -> p (a w)", p=P)

            pending_tail = [None]

            for hp in range(2):          # query half-pass (1024 queries)
                q0 = hp * 1024
                acc = psa.tile([9, 1024], F32, tag="acc")
                esbs = {}
                LAG = int(os.environ.get("OFFLAG", "2"))
                cons_seq = []
                for step in range(G + LAG):
                    lst = []
                    kpa = step - 2
                    if 0 <= kpa < G and kpa not in OFF_ALL[hp]:
                        lst.append(kpa)
                    kpo = step - LAG
                    if 0 <= kpo < G and kpo in OFF_ALL[hp]:
                        lst.append(kpo)
                    cons_seq.append(lst)
                flat = [kp for lst in cons_seq for kp in lst]
                first_kp, last_kp = flat[0], flat[-1]
                for kt in range(G + LAG):
                    if hp == 0 and kt == 2:
                        emit_tp_groups(8, 12, dma=True)
                    if hp == 0 and kt == 6:
                        emit_tp_groups(12, 16, dma=True)
                    if hp == 0 and kt == 10:
                        emit_tp_block(2, mode="dve")
                    if hp == 0 and kt == 12:
                        emit_tp_block(3, mode="dve")
                    if hp == 1 and kt == 2 and pending_tail[0] is not None:
                        pending_tail[0]()    # pass-0 tail, amortized here
                        pending_tail[0] = None
                    if kt < G:
                        # j-split eps tiles: [128,512] x 4 bufs gives each
                        # consumer lane (ACT/DVE/Pool) its own double-buffer
                        # so eps production stays off every lane's critical
                        # path.
                        for j in range(2):
                            eps = psb.tile([P, 512], F32, tag="big", bufs=4)
                            nc.tensor.matmul(
                                eps[:], Gf4[:, kt],
                                Ffq[:, :, q0 + j * 512:q0 + (j + 1) * 512],
                                start=True, stop=True, perf_mode=DR)
                            esb = epool.tile([P, 512], F32R, tag="e",
                                             bufs=12)
                            if 2 * kt + j in OFFJ[hp]:
                                nc.vector.tensor_scalar(
                                    esb[:], eps[:], LIN_A, LIN_C,
                                    op0=AL.mult, op1=AL.add)
                            else:
                                nc.scalar.activation(esb[:], eps[:],
                                                     ACTF.Exp)
                            esbs[(kt, j)] = esb
                    for kp in cons_seq[kt]:
                        vt = vaug_r
                        for j in range(2):
                            esb = esbs.pop((kp, j))
                            nc.tensor.matmul(
                                acc[:, j * 512:(j + 1) * 512],
                                vt[:, kp * 9:(kp + 1) * 9], esb[:],
                                start=(kp == first_kp),
                                stop=(kp == last_kp))

                # ---- tail for this half; pass-0's is deferred into the
                # middle of pass-1's loop so it doesn't stall the exp
                # stream.  The two j halves (= token-group quartets) run
                # as independent chains on DVE and Pool, each issuing its
                # own output DMA as soon as its groups are divided.
                def make_tail(hp, acc, fin_tag="small"):
                    def emit():
                        numden = sb.tile([9, 1024], F32R, tag="numden",
                                         bufs=2, name=f"numden{hp}")
                        tailt = psb.tile([P, 8 * 9], F32, tag="small",
                                         bufs=2, name=f"tailt{hp}")
                        ft3 = ftok[:].rearrange("p (a e) -> p a e", a=G)
                        ot3 = outt[:].rearrange("p (a e) -> p a e", a=G)
                        # hp0's tail is amortized mid-stream where ACT is
                        # the pacing lane -> both j chains on DVE; hp1's
                        # tail runs after ACT's last exp -> j1 on ACT so
                        # the two chains parallelize at the very end.
                        for j in range(2):
                            act_j = (j == 1 and hp == 1)
                            if not act_j:
                                # fold the quadratic's +GAM term in here:
                                # numden = acc + GAM*sum_off(vaug) (per row)
                                nc.vector.tensor_scalar(
                                    numden[:, j * 512:(j + 1) * 512],
                                    acc[:, j * 512:(j + 1) * 512],
                                    g9sb[hp][:], None, op0=AL.add)
                            else:
                                nc.scalar.activation(
                                    numden[:, 512:1024], acc[:, 512:1024],
                                    ACTF.Identity, bias=g9sb[hp][:])
                            fin_ps = psb.tile([9, 512], F32, tag="small",
                                              bufs=2, name=f"finps{hp}{j}")
                            nc.tensor.matmul(
                                fin_ps[:], w9_sb[:],
                                numden[:, j * 512:(j + 1) * 512],
                                start=True, stop=True)
                            fin_sb = sb.tile([9, 512], F32, tag="fin",
                                             bufs=2, name=f"finsb{hp}{j}")
                            if not act_j:
                                nc.vector.tensor_copy(fin_sb[:], fin_ps[:])
                            else:
                                nc.scalar.copy(fin_sb[:], fin_ps[:])
                            for aa in range(4):
                                a = j * 4 + aa
                                nc.tensor.transpose(
                                    tailt[:, a * 9:(a + 1) * 9],
                                    fin_sb[:, aa * P:(aa + 1) * P],
                                    ident[0:9, 0:9])
                            js = slice(hp * 72 + j * 36, hp * 72 + j * 36
                                       + 36)
                            if not act_j:
                                nc.vector.tensor_copy(
                                    ftok[:, js],
                                    tailt[:, j * 36:j * 36 + 36])
                            else:
                                nc.scalar.copy(
                                    ftok[:, js],
                                    tailt[:, j * 36:j * 36 + 36])
                            a0 = hp * 8 + j * 4
                            nc.vector.reciprocal(
                                recip[:, a0:a0 + 4].unsqueeze(2),
                                ft3[:, a0:a0 + 4, 8:9])
                            nc.vector.tensor_tensor(
                                ot3[:, a0:a0 + 4, :],
                                ft3[:, a0:a0 + 4, 0:8],
                                recip[:, a0:a0 + 4].unsqueeze(2)
                                .broadcast_to((P, 4, E)), op=AL.mult)
                            nc.sync.dma_start(
                                out_v[:, (hp * 2 + j) * 32:
                                      (hp * 2 + j) * 32 + 32],
                                outt[:, (hp * 2 + j) * 32:
                                     (hp * 2 + j) * 32 + 32])
                    return emit

                if hp == 0:
                    pending_tail[0] = make_tail(hp, acc)
                else:
                    make_tail(hp, acc)()

    nc.compile()
    return nc


def get_nc(reps=1):
    if reps not in _NC_CACHE:
        _NC_CACHE[reps] = _build_nc(reps)
    return _NC_CACHE[reps]


def kernel(x, phi_q, phi_k, phi_v, W, b, **_unused):
    x = np.ascontiguousarray(np.asarray(x, dtype=np.float32))
    W = np.asarray(W, dtype=np.float32)
    bb = np.asarray(b, dtype=np.float32)
    w9 = np.zeros((9, 9), np.float32)
    w9[0:8, 0:8] = W.T          # lhsT[d, e] = W[e, d]
    w9[8, 0:8] = bb             # bias enters as b * den
    w9[8, 8] = 1.0              # denominator passthrough
    phis = np.stack([phi_q, phi_k, phi_v]).astype(np.float32)
    crow = _make_crow()

    nc = get_nc()
    in_maps = [{"x": x[i], "w9": w9, "phis": phis, "crow": crow}
               for i in range(B)]
    res = run_bass_kernel_spmd(nc, in_maps, list(range(B)))
    return np.stack([res.results[i]["out"] for i in range(B)])



# revision 7
# speedup vs baseline: 2.1493x; 2.1493x over previous
"""Trainium2 Bass kernel for nn_MultiHeadAttentionQuantum — linear
attention via an exact rank-97 kernel expansion (no [S,S] materialization).

Math:
  - _qlayer(x, phi)[t, w] reduces to prefix products of cos(x+phi):
      out[t, w] = prod_{j<=w} cos(x[t,j]+phi[j])   (w >= 1)
      out[t, 0] = prod_{j=1..7} cos(x[t,j]+phi[j])
  - QuantumKernel sim factorizes:  sim[i,j] = prod_{w<4} cos((q_iw-k_jw)/2)
      sim   = F1 . G1, rank 16: products over wires of {cos(z/2), sin(z/2)}
      sim^2 = prod (1 + cos q cos k + sin q sin k)/2 = F2 . G2 / 16,
              rank 81: products over wires of {1, cos z, sin z}
  - sim in [0.2475, 1] empirically (>= cos(1)^4 analytically); exp(sim)
    is replaced by the degree-2 minimax fit on [0.2, 1]:
      exp(s) ~= C0 + C1 s + C2 s^2        (rel err 2.7e-3; softmax
    normalization cancels the common scale, measured end-to-end ~5e-3)
  - => E = exp(sim) is linear in 97 separable features:
      Phi_i = [F1(q_i) | F2(q_i)],  Psi_j = [G1(k_j) | G2(k_j)]
      E_ij  = sum_f coef_f Phi_if Psi_jf
      coef  = [C1 x16 | C2/16 x81],  coef[16] += C0  (feature 16 == 1)
    attention output = (E @ [v|1]) -> divide by last col -> @ W.T + b.
    All of it collapses to: M = sum_g Psi_g.T @ vaug_g  [97, 9],
    M2 = (M.T)^T @ w9 scaled by coef, fin = M2.T @ Phi.T [9, S],
    out = fin[0:8]/fin[8].  The S x S matrix never exists; total work is
    O(S * 97) elementwise + tiny matmuls.

Sharding: data-parallel over batch B=8, one batch element per NeuronCore,
no collectives. Full inputs in, full output out; host only slices/stacks.

Layout per core ("linear split"): SBUF partition p holds tokens
16p..16p+15 (contiguous 512B DMA lines both directions).

Engine split: DVE runs the q/k front-end chain and q-side (Phi) features;
Pool runs the v chain and k-side (Psi) features; ACT runs the 5 Sins
(cos chains + half/full-angle cos/sin of the qlayer outputs) and shares
PSUM->SBUF copies with DVE; PE does 16 bf16 feature transposes, the
M/M2/fin matmuls and the tail transposes.
"""
import os
import numpy as np

import concourse.bass as bass
import concourse.tile as tile
from concourse import bacc, mybir
from concourse.bass_utils import run_bass_kernel_spmd
from concourse.masks import make_identity

F32 = mybir.dt.float32
F32R = mybir.dt.float32r
BF16 = mybir.dt.bfloat16
AL = mybir.AluOpType
ACTF = mybir.ActivationFunctionType

B, S, E = 8, 2048, 8
P = 128          # SBUF partitions
G = 16           # token groups per partition (S / P)
NF = 97          # feature rank: 16 (half-angle) + 81 ({1,cos,sin})
MAGIC = 12582912.0           # 1.5 * 2**23: fp32 round-to-nearest trick
TWO_PI = float(2.0 * np.pi)
HALF_PI = float(0.5 * np.pi)

# degree-2 minimax (relative) fit of exp(s) on s in [0.2, 1.0]
C0 = 1.03344241
C1 = 0.77567233
C2 = 0.90192989

_NC_CACHE = {}


def _make_coef():
    coef = np.empty((NF, 1), np.float32)
    coef[0:16] = C1
    coef[16:NF] = C2 / 16.0
    coef[16] += C0          # F2 feature 0 is identically 1
    return coef


def _cos_chain(nc, work, x_bc, phi_bc, n, tagp, eng):
    """cos(x + phi) for n stacked phi-chains over G token groups.
    x_bc / phi_bc: [P, n, G, E] views (stride-0 broadcasts allowed).
    Elementwise ops on `eng`; the Sin runs on ACT. Returns c [P, n*G*E]."""
    W = n * G * E
    psi = work.tile([P, W], F32, tag=f"psi{tagp}")
    eng.tensor_tensor(
        psi[:].rearrange("p (n a w) -> p n a w", n=n, a=G), x_bc, phi_bc,
        op=AL.add)
    # range-reduce psi to [-pi, pi]:  psi - 2pi*round(psi/2pi)
    t1 = work.tile([P, W], F32, tag=f"t1{tagp}")
    eng.tensor_scalar(t1[:], psi[:], float(1.0 / TWO_PI), MAGIC,
                      op0=AL.mult, op1=AL.add)
    t2 = work.tile([P, W], F32, tag=f"t2{tagp}")
    eng.tensor_scalar(t2[:], t1[:], MAGIC, TWO_PI,
                      op0=AL.subtract, op1=AL.mult)
    red = work.tile([P, W], F32, tag=f"red{tagp}")
    eng.tensor_tensor(red[:], psi[:], t2[:], op=AL.subtract)
    c = work.tile([P, W], F32, tag=f"c{tagp}")
    nc.scalar.activation(c[:], red[:], ACTF.Sin)  # sin(x+phi+pi/2)=cos(x+phi)
    return c


def _prefix_products(nc, work, c3, n, tagp, eng):
    """u[j] = c[j-1]*c[j] (j>=1, u[0]=c[0]); v[j] = prod c[max(0,j-3)..j],
    per chain/group. c3: [P, n, G, 8] view."""
    u = work.tile([P, n * G * 8], F32, tag=f"u{tagp}")
    u3 = u[:].rearrange("p (n a w) -> p n a w", n=n, a=G)
    eng.tensor_copy(u3[:, :, :, 0:1], c3[:, :, :, 0:1])
    eng.tensor_tensor(u3[:, :, :, 1:8], c3[:, :, :, 1:8],
                      c3[:, :, :, 0:7], op=AL.mult)
    v = work.tile([P, n * G * 8], F32, tag=f"v{tagp}")
    v3 = v[:].rearrange("p (n a w) -> p n a w", n=n, a=G)
    eng.tensor_copy(v3[:, :, :, 0:2], u3[:, :, :, 0:2])
    eng.tensor_tensor(v3[:, :, :, 2:8], u3[:, :, :, 2:8],
                      u3[:, :, :, 0:6], op=AL.mult)
    return u3, v3


def _suffix1(nc, work, u3, c3, out1, n, tagp, eng):
    """out1 [P, n, G, 1] <- prod c[1..7] = u2*u4*u6*c7."""
    ta = work.tile([P, n * G], F32, tag=f"s1a{tagp}")
    ta3 = ta[:].rearrange("p (n a) -> p n a", n=n).unsqueeze(3)
    eng.tensor_tensor(ta3, u3[:, :, :, 2:3], u3[:, :, :, 4:5],
                      op=AL.mult)
    tb = work.tile([P, n * G], F32, tag=f"s1b{tagp}")
    tb3 = tb[:].rearrange("p (n a) -> p n a", n=n).unsqueeze(3)
    eng.tensor_tensor(tb3, u3[:, :, :, 6:7], c3[:, :, :, 7:8],
                      op=AL.mult)
    eng.tensor_tensor(out1, ta3, tb3, op=AL.mult)


def _build_nc(reps=1):
    nc = bacc.Bacc("TRN2", target_bir_lowering=False, debug=False,
                   num_devices=B)
    x_d = nc.dram_tensor("x", [S, E], F32, kind="ExternalInput").ap()
    w9_d = nc.dram_tensor("w9", [9, 9], F32, kind="ExternalInput").ap()
    phis_d = nc.dram_tensor("phis", [3, E], F32, kind="ExternalInput").ap()
    coef_d = nc.dram_tensor("coef", [NF, 1], F32, kind="ExternalInput").ap()
    out_d = nc.dram_tensor("out", [S, E], F32, kind="ExternalOutput").ap()

    with tile.TileContext(nc) as tc:
        with (
            tc.tile_pool(name="sb", bufs=1) as sb,
            tc.tile_pool(name="work", bufs=2) as work,
            tc.tile_pool(name="psb", bufs=2, space="PSUM") as psb,
        ):
          for _rep in range(reps):
            # ---- trig-table prefetch: a dummy Sin as the very first ACT
            # op loads the trig table while the x DMA is still in flight --
            tw0 = sb.tile([1, 1], F32, tag="tw0")
            nc.gpsimd.memset(tw0[:], 0.0)
            tw1 = sb.tile([1, 1], F32, tag="tw1")
            nc.scalar.activation(tw1[:], tw0[:], ACTF.Sin)

            # ---- loads & constants (x first: it gates everything) ----
            x_sb = sb.tile([P, P], F32, tag="x")
            nc.sync.dma_start(
                x_sb[:], x_d.rearrange("(p a) w -> p (a w)", p=P))
            phib = sb.tile([P, 3 * E], F32, tag="phib")
            nc.sync.dma_start(
                phib[:],
                phis_d.rearrange("n w -> (n w)").unsqueeze(0)
                .broadcast_to((P, 3 * E)))
            w9_ld = sb.tile([9, 9], F32, tag="w9ld")
            nc.sync.dma_start(w9_ld[:], w9_d[:])
            coef_sb = sb.tile([NF, 1], F32, tag="coef")
            nc.sync.dma_start(coef_sb[:], coef_d[:])
            ident = sb.tile([P, P], F32, tag="ident")
            make_identity(nc, ident[:])
            ident8 = sb.tile([P, P], BF16, tag="ident8")
            make_identity(nc, ident8[:])
            half_pi = sb.tile([P, 1], F32, tag="half_pi_const")
            nc.vector.memset(half_pi[:], HALF_PI)
            phibs = sb.tile([P, 3 * E], F32, tag="phibs")
            nc.vector.tensor_scalar(phibs[:], phib[:], HALF_PI, None,
                                    op0=AL.add)
            phibs3 = phibs[:].rearrange("p (n w) -> p n w", n=3)

            # ---- PE warm-up: dummy transposes keep the PE p-state/HAM
            # ramping while the front-end chain runs ----
            pewarm = psb.tile([P, P], BF16, tag="small", bufs=2)
            for _ in range(int(os.environ.get('PEWARM', '6'))):
                nc.tensor.transpose(pewarm[:], ident8[:], ident8[:])

            # ---- front-end: q+k cos chain on DVE, v chain on Pool ----
            x3 = x_sb[:].rearrange("p (a w) -> p a w", a=G)
            x_bc = x3.unsqueeze(1).broadcast_to((P, 2, G, E))
            phiqk = phibs3[:, 0:2, :].unsqueeze(2).broadcast_to((P, 2, G, E))
            c_qk = _cos_chain(nc, work, x_bc, phiqk, 2, "A", nc.vector)
            xv = x3.unsqueeze(1).broadcast_to((P, 1, G, E))
            phiv = phibs3[:, 2:3, :].unsqueeze(2).broadcast_to((P, 1, G, E))
            cv = _cos_chain(nc, work, xv, phiv, 1, "v", nc.gpsimd)

            warm2 = psb.tile([P, P], F32, tag="small", bufs=2, name="warm2")
            for _ in range(int(os.environ.get('PEWARM2', '4'))):
                nc.tensor.transpose(warm2[:], c_qk[:, 0:P], ident[:])

            # ---- qlayer prefix products -> z4 [P, 2, G, 4] (q,k wires
            # 0..3) on DVE ----
            cqk3 = c_qk[:].rearrange("p (n a w) -> p n a w", n=2, a=G)
            u3, v3 = _prefix_products(nc, work, cqk3, 2, "A", nc.vector)
            z4 = sb.tile([P, 2 * G * 4], F32, tag="z4")
            z44 = z4[:].rearrange("p (n a w) -> p n a w", n=2, a=G)
            nc.vector.tensor_copy(z44[:, :, :, 1:4], v3[:, :, :, 1:4])
            _suffix1(nc, work, u3, cqk3, z44[:, :, :, 0:1], 2, "A",
                     nc.vector)

            # ---- the 4 feature Sins: half-angle cos/sin (F1) and
            # full-angle cos/sin (F2) of z4, all on ACT ----
            # cs5 [P, b(2: cos/sin), n(2: q/k), G, 4]
            cs = sb.tile([P, 2 * 2 * G * 4], F32, tag="cs")
            cs5 = cs[:].rearrange("p (b n a w) -> p b n a w", b=2, n=2, a=G)
            nc.scalar.activation(cs5[:, 0], z44, ACTF.Sin,
                                 bias=half_pi[:], scale=0.5)
            nc.scalar.activation(cs5[:, 1], z44, ACTF.Sin, scale=0.5)
            # czsz [P, n, cz/sz, G, 4]
            czsz = sb.tile([P, 2 * 2 * G * 4], F32, tag="czsz")
            czsz5 = czsz[:].rearrange("p (n b a w) -> p n b a w", n=2, b=2,
                                      a=G)
            nc.scalar.activation(czsz5[:, :, 0], z44, ACTF.Sin,
                                 bias=half_pi[:])
            nc.scalar.activation(czsz5[:, :, 1], z44, ACTF.Sin)

            # ---- qlayer for v on Pool -> vaug [P, G, 9] (col 8 = 1) ----
            cv3 = cv[:].rearrange("p (n a w) -> p n a w", n=1, a=G)
            uv3, vv3 = _prefix_products(nc, work, cv3, 1, "v", nc.gpsimd)
            vaug = sb.tile([P, G * 9], F32, tag="vaug")
            nc.gpsimd.memset(vaug[:], 1.0)          # col 8 of each group = 1
            va4 = vaug[:].rearrange("p (a w) -> p a w", a=G).unsqueeze(1)
            nc.gpsimd.tensor_copy(va4[:, :, :, 1:4], vv3[:, :, :, 1:4])
            nc.gpsimd.tensor_tensor(va4[:, :, :, 4:8], vv3[:, :, :, 4:8],
                                    vv3[:, :, :, 0:4], op=AL.mult)
            _suffix1(nc, work, uv3, cv3, va4[:, :, :, 0:1], 1, "v",
                     nc.gpsimd)

            # ---- features ----
            # Phi (q side, bf16, will be PE-transposed) / Psi (k side, f32r,
            # direct lhsT for the M matmuls): [P, G, 97] token-major.
            phi_f = sb.tile([P, G * NF], BF16, tag="phi_f")
            psi_f = sb.tile([P, G * NF], F32, tag="psi_f")
            phi3 = phi_f[:].rearrange("p (a f) -> p a f", a=G)
            psi3 = psi_f[:].rearrange("p (a f) -> p a f", a=G)

            # F1: outer products of half-angle cos/sin (both sides at once)
            def sel(w):
                return cs5[:, :, :, :, w:w + 1].squeeze(4).transpose(
                    [0, 2, 3, 1])        # [P, n, G, b]

            a01 = work.tile([P, 2 * G * 4], F32, tag="a01")
            nc.vector.tensor_tensor(
                a01[:].rearrange("p (n a b1 b0) -> p n a b1 b0",
                                 n=2, a=G, b1=2),
                sel(0).unsqueeze(3).broadcast_to((P, 2, G, 2, 2)),
                sel(1).unsqueeze(4).broadcast_to((P, 2, G, 2, 2)),
                op=AL.mult)
            a23 = work.tile([P, 2 * G * 4], F32, tag="a23")
            nc.vector.tensor_tensor(
                a23[:].rearrange("p (n a b3 b2) -> p n a b3 b2",
                                 n=2, a=G, b3=2),
                sel(2).unsqueeze(3).broadcast_to((P, 2, G, 2, 2)),
                sel(3).unsqueeze(4).broadcast_to((P, 2, G, 2, 2)),
                op=AL.mult)
            a014 = a01[:].rearrange("p (n a lo) -> p n a lo", n=2, a=G)
            a234 = a23[:].rearrange("p (n a hi) -> p n a hi", n=2, a=G)

            def emit_f1(side, out3, eng):
                eng.tensor_tensor(
                    out3[:, :, 0:16].rearrange("p a (hi lo) -> p a hi lo",
                                               hi=4),
                    a014[:, side].unsqueeze(2).broadcast_to((P, G, 4, 4)),
                    a234[:, side].unsqueeze(3).broadcast_to((P, G, 4, 4)),
                    op=AL.mult)

            emit_f1(0, phi3, nc.vector)
            emit_f1(1, psi3, nc.gpsimd)

            # F2: t01/t23 = outer({1,cz,sz}_w0, {1,cz,sz}_w1) per side
            # czsz pair view for wire w: [P, n, G, 2(cz/sz)]
            def zw(w):
                return czsz5[:, :, :, :, w].transpose([0, 1, 3, 2])

            def emit_t(w0, w1, tag, eng):
                t = work.tile([P, 2 * G * 9], F32, tag=tag)
                t4 = t[:].rearrange("p (n a i) -> p n a i", n=2, a=G)
                eng.memset(t4[:, :, :, 0:1], 1.0)
                eng.tensor_copy(
                    t4[:, :, :, 1:3].rearrange("p n a (i o) -> p n a i o",
                                               i=2),
                    zw(w1).unsqueeze(4))
                for i in range(2):   # {cz,sz}(w0) x {1,cz,sz}(w1); <=3 free
                    eng.tensor_tensor(
                        t4[:, :, :, 3 + 3 * i:6 + 3 * i],
                        zw(w0)[:, :, :, i:i + 1].broadcast_to((P, 2, G, 3)),
                        t4[:, :, :, 0:3],
                        op=AL.mult)
                return t4

            t01 = emit_t(0, 1, "t01", nc.vector)
            t23 = emit_t(2, 3, "t23", nc.gpsimd)

            def emit_f2(side, out3, a0, a1, eng):
                ag = a1 - a0
                eng.tensor_tensor(
                    out3[:, a0:a1, 16:NF].rearrange(
                        "p a (hi lo) -> p a hi lo", hi=9),
                    t23[:, side, a0:a1].unsqueeze(3).broadcast_to(
                        (P, ag, 9, 9)),
                    t01[:, side, a0:a1].unsqueeze(2).broadcast_to(
                        (P, ag, 9, 9)),
                    op=AL.mult)

            # chunked so PE transposes / M matmuls start on groups 0-7
            # while groups 8-15 are still being built
            emit_f2(1, psi3, 0, 8, nc.gpsimd)
            emit_f2(0, phi3, 0, 8, nc.vector)
            emit_f2(1, psi3, 8, 16, nc.gpsimd)
            emit_f2(0, phi3, 8, 16, nc.vector)

            # ---- Phi transposes (PE, bf16) + M matmul accumulation ----
            phiT = sb.tile([NF, S], BF16, tag="phiT")
            m_ps = psb.tile([NF, 9], F32, tag="m_ps", bufs=1)
            for blk in range(4):
                tp = psb.tile([NF, 4 * P], BF16, tag="tp", bufs=2,
                              name=f"tp{blk}")
                for gl in range(4):
                    g = blk * 4 + gl
                    nc.tensor.transpose(
                        tp[:, gl * P:(gl + 1) * P],
                        phi_f[:, g * NF:(g + 1) * NF], ident8[:])
                    nc.tensor.matmul(
                        m_ps[:], psi_f[:, g * NF:(g + 1) * NF],
                        vaug[:, g * 9:(g + 1) * 9],
                        start=(g == 0), stop=(g == G - 1))
                eng = nc.vector if blk % 2 == 0 else nc.scalar
                if blk % 2 == 0:
                    eng.tensor_copy(
                        phiT[:, blk * 4 * P:(blk + 1) * 4 * P], tp[:])
                else:
                    eng.copy(phiT[:, blk * 4 * P:(blk + 1) * 4 * P], tp[:])

            # ---- M -> M2 = (M.T).T @ w9, scaled by coef, in bf16 ----
            m_sb = sb.tile([NF, 9], F32, tag="m_sb")
            nc.vector.tensor_copy(m_sb[:], m_ps[:])
            mt_ps = psb.tile([9, NF], F32, tag="small", bufs=2,
                              name="mt_ps")
            nc.tensor.transpose(mt_ps[:], m_sb[:], ident[0:NF, 0:NF])
            mt_sb = sb.tile([9, NF], F32, tag="mt_sb")
            nc.scalar.copy(mt_sb[:], mt_ps[:])
            m2_ps = psb.tile([NF, 9], F32, tag="small", bufs=2,
                             name="m2_ps")
            nc.tensor.matmul(m2_ps[:], mt_sb[:], w9_ld[:],
                             start=True, stop=True)
            m2_sb = sb.tile([NF, 9], BF16, tag="m2_sb")
            nc.vector.tensor_tensor(
                m2_sb[:], m2_ps[:],
                coef_sb[:].broadcast_to((NF, 9)), op=AL.mult)

            # ---- fin chunks: [9, 512] = M2.T @ PhiT, then transpose,
            # divide by the denominator row and DMA out.  Chunks alternate
            # DVE / ACT so two chains run in parallel at the end. ----
            ftok = sb.tile([P, G * 9], F32, tag="ftok")
            recip = sb.tile([P, G], F32, tag="recip")
            outt = sb.tile([P, P], F32, tag="outt")
            out_v = out_d.rearrange("(p a) w -> p (a w)", p=P)
            ft3 = ftok[:].rearrange("p (a e) -> p a e", a=G)
            ot3 = outt[:].rearrange("p (a e) -> p a e", a=G)

            for c in range(4):
                fin_ps = psb.tile([9, 512], F32, tag="fin", bufs=2,
                                  name=f"fin{c}")
                nc.tensor.matmul(fin_ps[:], m2_sb[:],
                                 phiT[:, c * 512:(c + 1) * 512],
                                 start=True, stop=True)
                act_c = (c % 2 == 1)
                fin_sb = sb.tile([9, 512], F32, tag="finsb", bufs=2,
                                 name=f"finsb{c}")
                if act_c:
                    nc.scalar.copy(fin_sb[:], fin_ps[:])
                else:
                    nc.vector.tensor_copy(fin_sb[:], fin_ps[:])
                tailt = psb.tile([P, 4 * 9], F32, tag="small", bufs=2,
                                 name=f"tailt{c}")
                for aa in range(4):
                    nc.tensor.transpose(
                        tailt[:, aa * 9:(aa + 1) * 9],
                        fin_sb[:, aa * P:(aa + 1) * P],
                        ident[0:9, 0:9])
                a0 = c * 4
                js = slice(a0 * 9, a0 * 9 + 36)
                if act_c:
                    nc.scalar.copy(ftok[:, js], tailt[:])
                else:
                    nc.vector.tensor_copy(ftok[:, js], tailt[:])
                nc.vector.reciprocal(
                    recip[:, a0:a0 + 4].unsqueeze(2),
                    ft3[:, a0:a0 + 4, 8:9])
                nc.vector.tensor_tensor(
                    ot3[:, a0:a0 + 4, :],
                    ft3[:, a0:a0 + 4, 0:8],
                    recip[:, a0:a0 + 4].unsqueeze(2)
                    .broadcast_to((P, 4, E)), op=AL.mult)
                nc.sync.dma_start(
                    out_v[:, c * 32:c * 32 + 32],
                    outt[:, c * 32:c * 32 + 32])

    nc.compile()
    return nc


def get_nc(reps=1):
    if reps not in _NC_CACHE:
        _NC_CACHE[reps] = _build_nc(reps)
    return _NC_CACHE[reps]


def kernel(x, phi_q, phi_k, phi_v, W, b, **_unused):
    x = np.ascontiguousarray(np.asarray(x, dtype=np.float32))
    W = np.asarray(W, dtype=np.float32)
    bb = np.asarray(b, dtype=np.float32)
    w9 = np.zeros((9, 9), np.float32)
    w9[0:8, 0:8] = W.T          # rhs[d, e] = W[e, d]
    w9[8, 0:8] = bb             # bias enters as b * den
    w9[8, 8] = 1.0              # denominator passthrough
    phis = np.stack([phi_q, phi_k, phi_v]).astype(np.float32)
    coef = _make_coef()

    nc = get_nc()
    in_maps = [{"x": x[i], "w9": w9, "phis": phis, "coef": coef}
               for i in range(B)]
    res = run_bass_kernel_spmd(nc, in_maps, list(range(B)))
    return np.stack([res.results[i]["out"] for i in range(B)])
